# revision 8
# baseline (speedup 1.0000x reference)
"""Trainium2 Bass kernel for a 2-layer GAT (PyG-style) over a random graph.

Strategy (8 NeuronCores, SPMD single program):
  - Destination-partition the 100k nodes contiguously across 8 cores
    (12500 each).  Within a core, sort nodes by in-degree and process
    them in tiles of 128 (one node per SBUF partition), padding each
    tile's edge count to the tile max degree.  Tiles are batched into
    groups of equal padded degree with <=GROUP_SLOTS edge slots.
  - Per layer, a per-node record table (fp16) is built on-device,
    sharded, and AllGathered so every core holds the full table in its
    DRAM.  Edge messages are fetched with ONE batched indirect DMA per
    tile group (offset AP [128, S]) -- the per-instruction ~1us SWDGE
    fixed cost amortizes over all S slot columns.
  - Records are trimmed: layer 1 gathers h1[64]+a_src1[8] (a_dst1 is
    destination-side and kept resident in SBUF); layer 2 gathers
    z[40]+a_src2+a_dst2 where z = ELU(h1_agg) @ W2 is precomputed per
    node (linearity of the attention-weighted sum wrt W2).
  - Segment softmax uses max-subtraction so the whole edge pipeline
    (exp, weighted message, tree adds) runs in fp16.
  - log_softmax's ln() is deferred to one post-loop instruction over a
    resident [128, TILES] tile (avoids ACT table thrash), and the final
    output is written with a single DMA.

The kernel function accepts the FULL inputs and returns the FULL output.
"""

import sys

for _p in ("/opt/trn_rl_repo",):
    if _p not in sys.path:
        sys.path.insert(0, _p)

import numpy as np

# ---------------------------------------------------------------- constants
N = 100000
E = 3200000
F_IN = 128
H1 = 8           # layer-1 heads
C1 = 8           # layer-1 channels per head
HC1 = H1 * C1    # 64
C2 = 40          # layer-2 channels (1 head)
NCORES = 8
NPC = N // NCORES            # 12500 nodes per core
TILES = (NPC + 127) // 128   # 98
NPAD = TILES * 128           # 12544
SHARD = 12800                # table rows per core (25 * 512), >= NPAD
REC1 = 72                    # fp16 words: h1[64] | a_src1[8]
REC2 = 42                    # fp16 words: z[40] | a_src2 | a_dst2
NEG_SLOPE = 0.2
EPS = 1e-16
PAD_LOGIT = -30000.0         # a_src for pad rows -> exp == 0 after max-sub
GROUP_SLOTS = 128            # max padded edge slots per tile group


# ---------------------------------------------------------------- host prep
def _preprocess(edge_index):
    """Build the per-core tile schedule and gather-offset tables."""
    src = np.ascontiguousarray(edge_index[0]).astype(np.int64)
    dst = np.ascontiguousarray(edge_index[1]).astype(np.int64)
    loops = np.arange(N, dtype=np.int64)
    src = np.concatenate([loops, src])
    dst = np.concatenate([loops, dst])

    deg = np.bincount(dst, minlength=N)

    perms = np.empty((NCORES, NPC), np.int64)   # node ids, degree-desc per core
    pos = np.empty(N, np.int64)                 # position of node in its core perm
    for c in range(NCORES):
        nodes = np.arange(c * NPC, (c + 1) * NPC)
        order = np.argsort(-deg[nodes], kind="stable")
        perm = nodes[order]
        perms[c] = perm
        pos[perm] = np.arange(NPC)

    # chunk-major table rows: [chunk0: 8 cores x 6144][chunk1: 8 x 6656]
    # so each partial bounce buffer AllGathers into one contiguous range
    HCHUNK = 6144
    core_of = np.arange(N) // NPC
    row_of = np.where(
        pos < HCHUNK,
        core_of * HCHUNK + pos,
        NCORES * HCHUNK + core_of * (SHARD - HCHUNK) + (pos - HCHUNK))

    # tab2 uses its own chunking [8 x 11264][8 x 1536]: the mid-stream
    # AllGather (fired late in layer 1) covers almost everything and the
    # blocking tail collective shrinks to 1536 rows per core
    H2 = 11264
    row2_of = np.where(
        pos < H2,
        core_of * H2 + pos,
        NCORES * H2 + core_of * (SHARD - H2) + (pos - H2))

    # common tile degree schedule (max over cores per tile index)
    degs_sorted = np.zeros((NCORES, NPAD), np.int64)
    degs_sorted[:, :NPC] = deg[perms]
    dtile = degs_sorted.reshape(NCORES, TILES, 128).max(axis=2).max(axis=0)
    dtile = np.maximum(dtile, 1)

    # group consecutive tiles: equal padded degree, <= GROUP_SLOTS slots
    groups = []  # (tile_start, ntiles, dbar)
    t = 0
    while t < TILES:
        dbar = int(dtile[t])
        j = max(1, GROUP_SLOTS // dbar)
        j = min(j, TILES - t)
        groups.append((t, j, dbar))
        t += j

    # per-tile padded degree after grouping and column offsets
    dpad = np.empty(TILES, np.int64)
    for (t0, j, dbar) in groups:
        dpad[t0:t0 + j] = dbar
    col0 = np.zeros(TILES + 1, np.int64)
    np.cumsum(dpad, out=col0[1:])
    sd = int(col0[-1])

    # offsets[c, p, col]: table row gathered into slot (tile, p, d)
    offs = np.empty((NCORES, 128, sd), np.int32)
    offs2 = np.empty((NCORES, 128, sd), np.int32)
    for c in range(NCORES):
        offs[c] = (NCORES * 6144 + c * (SHARD - 6144)
                   + (SHARD - 6144 - 1))   # pad row (a_src == PAD_LOGIT)
        offs2[c] = (NCORES * 11264 + c * (SHARD - 11264)
                    + (SHARD - 11264 - 1))

    order_e = np.argsort(dst, kind="stable")
    ds = dst[order_e]
    sv = src[order_e]
    counts = np.bincount(ds, minlength=N)
    seg_start = np.concatenate([[0], np.cumsum(counts)[:-1]])
    rank = np.arange(ds.shape[0]) - seg_start[ds]

    c_e = ds // NPC
    pos_e = pos[ds]
    p_e = pos_e % 128
    t_e = pos_e // 128
    col_e = col0[t_e] + rank
    offs[c_e, p_e, col_e] = row_of[sv].astype(np.int32)
    offs2[c_e, p_e, col_e] = row2_of[sv].astype(np.int32)

    return dict(perms=perms, dtile=dpad, groups=groups, col0=col0, sd=sd,
                offs=offs, offs2=offs2, deg=deg)


def _make_inputs(x, W1, att_src1, att_dst1, b1, W2, att_src2, att_dst2, b2, prep):
    """Per-core input maps for the SPMD kernel."""
    f16 = np.float16
    f32 = np.float32

    # layer-1 attention folded into x-side weights: a1 = x @ (W1 @ Atil)
    Atil = np.zeros((HC1, 16), f32)
    for h in range(H1):
        Atil[h * C1:(h + 1) * C1, h] = att_src1[h]
        Atil[h * C1:(h + 1) * C1, 8 + h] = att_dst1[h]
    # combined dense weights: [W1 | W1@Atil] -> [128, 80]
    w1a = np.concatenate([W1.astype(f32),
                          W1.astype(f32) @ Atil], axis=1).astype(f16)

    # layer-2: z = h2 @ W2 plus attention scalars in one matmul
    ws2 = W2.astype(f32) @ att_src2[0].astype(f32)   # [64]
    wd2 = W2.astype(f32) @ att_dst2[0].astype(f32)   # [64]
    w2aug = np.concatenate([W2.astype(f32), ws2[:, None], wd2[:, None]],
                           axis=1).astype(f16)       # [64, 42]

    common = {
        "w1a": w1a,                                       # [128, 80] fp16
        "w2aug": w2aug,                                   # [64, 42] fp16
        "b1r": np.tile(b1.astype(f32), (128, 1)),         # [128, 64]
        "b2r": np.tile(b2.astype(f32), (128, 1)),         # [128, 40]
    }
    pr1 = np.zeros((128, REC1), f16)
    pr1[:, 64:72] = PAD_LOGIT
    pr2 = np.zeros((128, REC2), f16)
    pr2[:, 40] = PAD_LOGIT
    common["padrec1"] = pr1
    common["padrec2"] = pr2

    in_maps = []
    for c in range(NCORES):
        xt = np.zeros((SHARD, F_IN), f16)
        xt[:NPC] = x[prep["perms"][c]].astype(f16)
        m = dict(common)
        m["xts"] = np.ascontiguousarray(xt.T)               # [128, SHARD] fp16
        m["offs"] = prep["offs"][c]                         # [128, sd] int32
        m["offs2"] = prep["offs2"][c]
        in_maps.append(m)
    return in_maps


# ---------------------------------------------------------------- bass build
def _build(prep):
    from concourse import bass, bacc, mybir
    from concourse.tile import TileContext
    from concourse.masks import make_identity

    f16 = mybir.dt.float16
    f32 = mybir.dt.float32
    i32 = mybir.dt.int32
    OP = mybir.AluOpType
    ACTF = mybir.ActivationFunctionType
    AX = mybir.AxisListType

    groups = prep["groups"]
    col0 = prep["col0"]
    sd = prep["sd"]

    nc = bacc.Bacc("TRN2", target_bir_lowering=False, debug=False,
                   num_devices=NCORES)

    xts = nc.declare_dram_parameter("xts", [128, SHARD], f16, isOutput=False)
    offs = nc.declare_dram_parameter("offs", [128, sd], i32, isOutput=False)
    offs2 = nc.declare_dram_parameter("offs2", [128, sd], i32, isOutput=False)
    w1a = nc.declare_dram_parameter("w1a", [128, 80], f16, isOutput=False)
    w2aug = nc.declare_dram_parameter("w2aug", [HC1, REC2], f16, isOutput=False)
    b1r = nc.declare_dram_parameter("b1r", [128, HC1], f32, isOutput=False)
    b2r = nc.declare_dram_parameter("b2r", [128, C2], f32, isOutput=False)
    padrec1 = nc.declare_dram_parameter("padrec1", [128, REC1], f16, isOutput=False)
    padrec2 = nc.declare_dram_parameter("padrec2", [128, REC2], f16, isOutput=False)
    outp = nc.declare_dram_parameter("out", [NPAD, C2], f32, isOutput=True)

    bnc1 = nc.dram_tensor("bounce1", [SHARD, REC1], f16)
    tab1 = nc.dram_tensor("table1", [NCORES * SHARD, REC1], f16, addr_space="Shared")
    bnc2 = nc.dram_tensor("bounce2", [SHARD, REC2], f16)
    tab2 = nc.dram_tensor("table2", [NCORES * SHARD, REC2], f16, addr_space="Shared")

    with TileContext(nc) as tc:
        with (
            tc.tile_pool(name="const", bufs=1) as cpool,
            tc.tile_pool(name="dense", bufs=3) as dpool,
            tc.tile_pool(name="gth", bufs=2) as gpool,
            tc.tile_pool(name="mbuf", bufs=2) as mpool,
            tc.tile_pool(name="small", bufs=3) as spool,
            tc.tile_pool(name="psum", bufs=2, space="PSUM") as ppool,
        ):
            # ---- resident constants
            w1s = cpool.tile([128, 80], f16)
            nc.sync.dma_start(out=w1s[:], in_=w1a[:])
            w2s = cpool.tile([HC1, REC2], f16)
            nc.sync.dma_start(out=w2s[:], in_=w2aug[:])
            b1s = cpool.tile([128, HC1], f32)
            nc.sync.dma_start(out=b1s[:], in_=b1r[:])
            b2s = cpool.tile([128, C2], f32)
            nc.sync.dma_start(out=b2s[:], in_=b2r[:])
            pr1s = cpool.tile([128, REC1], f16)
            nc.sync.dma_start(out=pr1s[:], in_=padrec1[:])
            pr2s = cpool.tile([128, REC2], f16)
            nc.sync.dma_start(out=pr2s[:], in_=padrec2[:])
            idn = cpool.tile([128, 128], f32)
            make_identity(nc, idn[:])
            offs_sb = cpool.tile([128, sd], i32)
            nc.sync.dma_start(out=offs_sb[:], in_=offs[:])

            # resident per-node destination-side data (never leaves SBUF)
            adst1_sb = cpool.tile([128, TILES * H1], f16)
            adst2_sb = cpool.tile([128, TILES], f16)
            sh_sb = cpool.tile([128, TILES * C2], f32)
            sm_sb = cpool.tile([128, TILES], f32)
            lg_sb = cpool.tile([128, TILES], f32)

            # ---- dense phase: per-node records for layer 1 (own shard)
            HC = 6144
            for k in range(SHARD // 512):
                xt = dpool.tile([128, 512], f16, tag="xt")
                nc.sync.dma_start(out=xt[:], in_=xts[:, k * 512:(k + 1) * 512])
                ps = ppool.tile([128, 320], f32, tag="psd")
                for j in range(4):
                    nc.tensor.matmul(ps[:, j * 80:(j + 1) * 80],
                                     lhsT=xt[:, j * 128:(j + 1) * 128],
                                     rhs=w1s[:], start=True, stop=True)
                rec4 = dpool.tile([128, 4 * REC1], f16, tag="rec")
                nc.vector.tensor_copy(
                    out=rec4[:].rearrange("p (j r) -> p j r", r=REC1),
                    in_=ps[:].rearrange("p (j q) -> p j q", q=80)[:, :, 0:REC1])
                nc.sync.dma_start(
                    out=bnc1[k * 512:(k + 1) * 512, :]
                        .rearrange("(j p) r -> p j r", p=128),
                    in_=rec4[:].rearrange("p (j r) -> p j r", r=REC1))
                na = min(TILES - k * 4, 4)
                if na > 0:
                    nc.vector.tensor_copy(
                        out=adst1_sb[:, k * 4 * H1:(k * 4 + na) * H1]
                            .rearrange("p (j h) -> p j h", h=H1),
                        in_=ps[:].rearrange("p (j q) -> p j q", q=80)
                            [:, 0:na, 72:80])
                if (k + 1) * 512 == HC:  # k == 11
                    # first half of the records is complete: AllGather it
                    # while the dense phase finishes the second half
                    nc.gpsimd.collective_compute(
                        "AllGather", OP.bypass,
                        replica_groups=[list(range(NCORES))],
                        ins=[bnc1[0:HC, :]], outs=[tab1[0:NCORES * HC, :]],
                    )
            # pad-row tail [NPAD, SHARD): overwrite after the dense loop
            for i in range((SHARD - NPAD) // 128):
                r0 = NPAD + i * 128
                nc.sync.dma_start(out=bnc1[r0:r0 + 128, :], in_=pr1s[:])
                nc.sync.dma_start(out=bnc2[r0:r0 + 128, :], in_=pr2s[:])

            nc.gpsimd.collective_compute(
                "AllGather", OP.bypass,
                replica_groups=[list(range(NCORES))],
                ins=[bnc1[HC:, :]], outs=[tab1[NCORES * HC:, :]],
            )

            # ---- layer 1 tile groups
            _ag2a_after = next(i for i, (t0_, J_, _d) in enumerate(groups)
                               if t0_ + J_ >= 11264 // 128)
            for _gi, (t0, J, D) in enumerate(groups):
                S = J * D      # edge slots in group
                g = gpool.tile([128, S * REC1], f16, tag="g")
                gv = g[:].rearrange("p (s r) -> p s r", r=REC1)
                for s in range(S):
                    if s % D == 0:
                        continue  # self column, direct-loaded below
                    nc.gpsimd.indirect_dma_start(
                        out=g[:, s * REC1:(s + 1) * REC1], out_offset=None,
                        in_=tab1[:],
                        in_offset=bass.IndirectOffsetOnAxis(
                            ap=offs_sb[:, col0[t0] + s:col0[t0] + s + 1],
                            axis=0),
                    )
                nc.sync.dma_start(
                    out=gv[:, :, 0:REC1]
                        .rearrange("p (j d) r -> p j d r", j=J)[:, :, 0, :],
                    in_=bnc1[t0 * 128:(t0 + J) * 128, :]
                        .rearrange("(j p) r -> p j r", p=128))

                # e = leaky(a_src[src] + a_dst[dst]); softmax-stable exp
                e = spool.tile([128, S * H1], f16, tag="e")
                nc.vector.tensor_tensor(
                    out=e[:].rearrange("p (j d h) -> p j d h", j=J, h=H1),
                    in0=gv[:, :, 64:72].rearrange("p (j d) h -> p j d h", j=J),
                    in1=adst1_sb[:, t0 * H1:(t0 + J) * H1]
                        .rearrange("p (j h) -> p j h", h=H1)
                        .unsqueeze(2).broadcast_to([128, J, D, H1]),
                    op=OP.add)
                el = spool.tile([128, S * H1], f16, tag="el")
                nc.vector.scalar_tensor_tensor(
                    out=el[:], in0=e[:], scalar=NEG_SLOPE, in1=e[:],
                    op0=OP.mult, op1=OP.max)
                mx = spool.tile([128, J * H1], f16, tag="mx")
                nc.vector.tensor_reduce(
                    out=mx[:],
                    in_=el[:].rearrange("p (j d h) -> p j h d", j=J, h=H1),
                    axis=AX.X, op=OP.max)
                es = spool.tile([128, S * H1], f16, tag="es")
                nc.vector.tensor_tensor(
                    out=es[:].rearrange("p (j d h) -> p j d h", j=J, h=H1),
                    in0=el[:].rearrange("p (j d h) -> p j d h", j=J, h=H1),
                    in1=mx[:].rearrange("p (j h) -> p j h", h=H1)
                        .unsqueeze(2).broadcast_to([128, J, D, H1]),
                    op=OP.subtract)
                ex = spool.tile([128, S * H1], f16, tag="ex")
                nc.scalar.activation(out=ex[:], in_=es[:], func=ACTF.Exp)

                den = spool.tile([128, J * H1], f32, tag="den")
                nc.vector.tensor_reduce(
                    out=den[:],
                    in_=ex[:].rearrange("p (j d h) -> p j h d", j=J, h=H1),
                    axis=AX.X, op=OP.add)

                m = mpool.tile([128, S * HC1], f16, tag="m")
                nc.vector.tensor_tensor(
                    out=m[:].rearrange("p (s h c) -> p s h c", h=H1, c=C1),
                    in0=gv[:, :, 0:HC1].rearrange("p s (h c) -> p s h c", h=H1),
                    in1=ex[:].rearrange("p (s h) -> p s h", h=H1)
                        .unsqueeze(3).broadcast_to([128, S, H1, C1]),
                    op=OP.mult)
                ms = spool.tile([128, J * HC1], f32, tag="ms")
                nc.vector.tensor_reduce(
                    out=ms[:],
                    in_=m[:].rearrange("p (j d c) -> p j c d", j=J, c=HC1),
                    axis=AX.X, op=OP.add)

                rc = spool.tile([128, J * H1], f32, tag="rc")
                nc.vector.tensor_scalar_add(out=rc[:], in0=den[:], scalar1=EPS)
                nc.vector.reciprocal(out=rc[:], in_=rc[:])

                o1 = spool.tile([128, J * HC1], f32, tag="o1")
                nc.vector.tensor_tensor(
                    out=o1[:].rearrange("p (j h c) -> p j h c", h=H1, c=C1),
                    in0=ms[:].rearrange("p (j h c) -> p j h c", h=H1, c=C1),
                    in1=rc[:].rearrange("p (j h) -> p j h", h=H1)
                        .unsqueeze(3).broadcast_to([128, J, H1, C1]),
                    op=OP.mult)
                # + b1 (broadcast rows pre-replicated on host)
                nc.vector.tensor_tensor(
                    out=o1[:].rearrange("p (j c) -> p j c", c=HC1),
                    in0=o1[:].rearrange("p (j c) -> p j c", c=HC1),
                    in1=b1s[:].unsqueeze(1).broadcast_to([128, J, HC1]),
                    op=OP.add)

                # ELU -> h2 (fp16)
                t1 = spool.tile([128, J * HC1], f32, tag="t1")
                nc.vector.tensor_scalar_min(out=t1[:], in0=o1[:], scalar1=0.0)
                nc.scalar.activation(out=t1[:], in_=t1[:], func=ACTF.Exp)
                h2 = spool.tile([128, J * HC1], f32, tag="h2")
                nc.vector.scalar_tensor_tensor(
                    out=h2[:], in0=t1[:], scalar=-1.0, in1=o1[:],
                    op0=OP.add, op1=OP.max)

                # z = h2 @ [W2 | ws2 | wd2] per tile (transpose trick)
                rec2 = spool.tile([128, J * REC2], f16, tag="rec2")
                for j in range(J):
                    pst = ppool.tile([HC1, 128], f32, tag="pst")
                    nc.tensor.transpose(out=pst[:],
                                        in_=h2[:, j * HC1:(j + 1) * HC1],
                                        identity=idn[:])
                    h2t = spool.tile([HC1, 128], f16, tag="h2t")
                    nc.vector.tensor_copy(out=h2t[:], in_=pst[:])
                    ps2 = ppool.tile([128, REC2], f32, tag="ps2")
                    nc.tensor.matmul(ps2[:], lhsT=h2t[:], rhs=w2s[:],
                                     start=True, stop=True)
                    nc.vector.tensor_copy(out=rec2[:, j * REC2:(j + 1) * REC2],
                                          in_=ps2[:])
                    nc.vector.tensor_copy(out=adst2_sb[:, t0 + j:t0 + j + 1],
                                          in_=ps2[:, 41:42])

                nc.sync.dma_start(
                    out=bnc2[t0 * 128:(t0 + J) * 128, :]
                        .rearrange("(j p) r -> p j r", p=128),
                    in_=rec2[:].rearrange("p (j r) -> p j r", r=REC2))
                if _gi == _ag2a_after:
                    # z records for tab2 chunk 0 (rows < 11264 = tiles
                    # 0..87) are complete: AllGather them under the
                    # remaining layer-1 groups
                    nc.gpsimd.collective_compute(
                        "AllGather", OP.bypass,
                        replica_groups=[list(range(NCORES))],
                        ins=[bnc2[0:11264, :]],
                        outs=[tab2[0:NCORES * 11264, :]],
                    )

            nc.gpsimd.collective_compute(
                "AllGather", OP.bypass,
                replica_groups=[list(range(NCORES))],
                ins=[bnc2[11264:, :]], outs=[tab2[NCORES * 11264:, :]],
            )
            # swap the offset table to tab2's row numbering for layer 2
            nc.sync.dma_start(out=offs_sb[:], in_=offs2[:])

            # ---- layer 2 tile groups
            for (t0, J, D) in groups:
                S = J * D
                g2 = gpool.tile([128, S * REC2], f16, tag="g2")
                g2v = g2[:].rearrange("p (s r) -> p s r", r=REC2)
                for s in range(S):
                    if s % D == 0:
                        continue  # self column, direct-loaded below
                    nc.gpsimd.indirect_dma_start(
                        out=g2[:, s * REC2:(s + 1) * REC2], out_offset=None,
                        in_=tab2[:],
                        in_offset=bass.IndirectOffsetOnAxis(
                            ap=offs_sb[:, col0[t0] + s:col0[t0] + s + 1],
                            axis=0),
                    )
                nc.sync.dma_start(
                    out=g2v[:, :, 0:REC2]
                        .rearrange("p (j d) r -> p j d r", j=J)[:, :, 0, :],
                    in_=bnc2[t0 * 128:(t0 + J) * 128, :]
                        .rearrange("(j p) r -> p j r", p=128))

                e2 = spool.tile([128, S], f16, tag="e")
                nc.vector.tensor_tensor(
                    out=e2[:].rearrange("p (j d) -> p j d", j=J),
                    in0=g2v[:, :, 40].rearrange("p (j d) -> p j d", j=J),
                    in1=adst2_sb[:, t0:t0 + J]
                        .unsqueeze(2).broadcast_to([128, J, D]),
                    op=OP.add)
                el2 = spool.tile([128, S], f16, tag="el")
                nc.vector.scalar_tensor_tensor(
                    out=el2[:], in0=e2[:], scalar=NEG_SLOPE, in1=e2[:],
                    op0=OP.mult, op1=OP.max)
                mx2 = spool.tile([128, J], f16, tag="mx")
                nc.vector.tensor_reduce(
                    out=mx2[:],
                    in_=el2[:].rearrange("p (j d) -> p j d", j=J),
                    axis=AX.X, op=OP.max)
                es2 = spool.tile([128, S], f16, tag="es")
                nc.vector.tensor_tensor(
                    out=es2[:].rearrange("p (j d) -> p j d", j=J),
                    in0=el2[:].rearrange("p (j d) -> p j d", j=J),
                    in1=mx2[:].unsqueeze(2).broadcast_to([128, J, D]),
                    op=OP.subtract)
                ex2 = spool.tile([128, S], f16, tag="ex")
                nc.scalar.activation(out=ex2[:], in_=es2[:], func=ACTF.Exp)

                den2 = spool.tile([128, J], f32, tag="den")
                nc.vector.tensor_reduce(
                    out=den2[:],
                    in_=ex2[:].rearrange("p (j d) -> p j d", j=J),
                    axis=AX.X, op=OP.add)

                m2 = mpool.tile([128, S * C2], f16, tag="m2")
                nc.vector.tensor_tensor(
                    out=m2[:].rearrange("p (s c) -> p s c", c=C2),
                    in0=g2v[:, :, 0:C2],
                    in1=ex2[:].unsqueeze(2).broadcast_to([128, S, C2]),
                    op=OP.mult)
                ms2 = spool.tile([128, J * C2], f32, tag="ms2")
                nc.vector.tensor_reduce(
                    out=ms2[:],
                    in_=m2[:].rearrange("p (j d c) -> p j c d", j=J, c=C2),
                    axis=AX.X, op=OP.add)

                rc2 = spool.tile([128, J], f32, tag="rc")
                nc.vector.tensor_scalar_add(out=rc2[:], in0=den2[:], scalar1=EPS)
                nc.vector.reciprocal(out=rc2[:], in_=rc2[:])

                z = spool.tile([128, J * C2], f32, tag="z")
                nc.vector.tensor_tensor(
                    out=z[:].rearrange("p (j c) -> p j c", c=C2),
                    in0=ms2[:].rearrange("p (j c) -> p j c", c=C2),
                    in1=rc2[:].unsqueeze(2).broadcast_to([128, J, C2]),
                    op=OP.mult)
                nc.vector.tensor_tensor(
                    out=z[:].rearrange("p (j c) -> p j c", c=C2),
                    in0=z[:].rearrange("p (j c) -> p j c", c=C2),
                    in1=b2s[:].unsqueeze(1).broadcast_to([128, J, C2]),
                    op=OP.add)

                # log-softmax part 1: shifted logits + sum(exp), ln deferred
                zmx = spool.tile([128, J], f32, tag="zmx")
                nc.vector.tensor_reduce(
                    out=zmx[:], in_=z[:].rearrange("p (j c) -> p j c", c=C2),
                    axis=AX.X, op=OP.max)
                nc.vector.tensor_tensor(
                    out=sh_sb[:, t0 * C2:(t0 + J) * C2]
                        .rearrange("p (j c) -> p j c", c=C2),
                    in0=z[:].rearrange("p (j c) -> p j c", c=C2),
                    in1=zmx[:].unsqueeze(2).broadcast_to([128, J, C2]),
                    op=OP.subtract)
                ee = spool.tile([128, J * C2], f32, tag="ee")
                nc.scalar.activation(
                    out=ee[:], in_=sh_sb[:, t0 * C2:(t0 + J) * C2],
                    func=ACTF.Exp)
                nc.vector.tensor_reduce(
                    out=sm_sb[:, t0:t0 + J],
                    in_=ee[:].rearrange("p (j c) -> p j c", c=C2),
                    axis=AX.X, op=OP.add)

            # ---- epilogue: ln once, subtract, single output DMA
            nc.scalar.activation(out=lg_sb[:], in_=sm_sb[:], func=ACTF.Ln)
            nc.vector.tensor_tensor(
                out=sh_sb[:].rearrange("p (t c) -> p t c", c=C2),
                in0=sh_sb[:].rearrange("p (t c) -> p t c", c=C2),
                in1=lg_sb[:].unsqueeze(2).broadcast_to([128, TILES, C2]),
                op=OP.subtract)
            nc.sync.dma_start(
                out=outp[:].rearrange("(t p) c -> p t c", p=128),
                in_=sh_sb[:].rearrange("p (t c) -> p t c", c=C2))

    nc.compile()
    return nc


# ---------------------------------------------------------------- entry
def kernel(x, edge_index, W1, att_src1, att_dst1, b1, W2, att_src2, att_dst2, b2,
           _debug_trace=False):
    from concourse.bass_utils import run_bass_kernel_spmd

    x = np.asarray(x)
    edge_index = np.asarray(edge_index)

    prep = _preprocess(edge_index)
    in_maps = _make_inputs(np.asarray(x, np.float32), np.asarray(W1, np.float32),
                           np.asarray(att_src1, np.float32),
                           np.asarray(att_dst1, np.float32),
                           np.asarray(b1, np.float32),
                           np.asarray(W2, np.float32),
                           np.asarray(att_src2, np.float32),
                           np.asarray(att_dst2, np.float32),
                           np.asarray(b2, np.float32), prep)
    nc = _build(prep)

    kw = {}
    if _debug_trace:
        import os
        td = "/tmp/gat_trace"
        os.makedirs(td, exist_ok=True)
        for f in os.listdir(td):
            os.unlink(os.path.join(td, f))
        kw["tmpdir"] = td
    res = run_bass_kernel_spmd(nc, in_maps, list(range(NCORES)),
                               trace=_debug_trace, **kw)
    out = np.empty((N, C2), np.float32)
    for c in range(NCORES):
        out[prep["perms"][c]] = np.asarray(res.results[c]["out"])[:NPC]
    kernel._last_results = res
    return out

